# revision 23
# baseline (speedup 1.0000x reference)
"""Trainium2 Bass kernel for nn_BidirectionalAttention.

reference math (per batch):
    sim = (q @ w_q)[None, :] + (ctx @ w_c)[:, None] + (ctx * w_m) @ q.T   # (C, Q)
    c2q = softmax(sim, axis=1) @ q                                        # (C, E)
    m   = max(sim, axis=1);  w = softmax(m, axis=0)                       # (C,)
    q2c = w @ ctx                                                         # (E,)

Strategy: data-parallel over batch across 8 cores (8 batches/core).
On device, per batch:
  stats orientation (C on partitions, Q on free):
    sim' = ctx @ (w_m*q).T + 1*qw^T  via PE matmuls (lhsT = ctxT chunks),
    with w_c appended as a 65th rhs column so cw = ctx@w_c rides along.
    rowmax (exact, incl. qw) and rowsum(exp(sim'-rowmax)) via free-dim reduces.
  transposed orientation (Q on partitions, C on free):
    simT = qmT.T @ ctxT  (PE), minus shift2 = rowmax + ln(rowsum) via a
    K=1 rank-1 matmul (shift2 delivered as row vectors through a PE
    transpose + small SBUF->SBUF reshape DMA), plus qw via ACT exp bias.
    ZTn = exp(...) is then directly the softmax^T -> c2qT = qN.T @ ZTn.
  q2c: u = exp(rowmax + cw - 125) in column form; q2c = (u^T ctx)/sum(u)
    via K=128 matvec matmuls against naturally-laid ctx, with the
    partition sum of u done by a ones-matmul.

Host pre-transposes context/question into the layouts the device wants and
re-assembles the transposed c2q output. All heavy FLOPs and all softmax
math run on device.
"""

import functools
import os
import sys

import numpy as np

sys.path.insert(0, "/opt/trn_rl_repo")

B, C, Q, E = 64, 1024, 64, 256
NCORES = 8
BPC = B // NCORES  # batches per core
P = 128
EJ = E // P  # 2 E-chunks
CK = C // P  # 8 C-chunks
Q2C_SHIFT = 125.0  # stabilization shift for the q2c softmax (see design notes)

# matmul dtype knobs (f32 = exact 4cyc/row; f32r = fp22 1cyc/row at N>=256)
_F32R = os.environ.get("KERNEL_F32R", "1") == "1"
_DEBUG = os.environ.get("KERNEL_DEBUG", "0") == "1"

LAST_EXEC_NS = None
LAST_TRACE = None


def _ensure_ntff_hook():
    """Provide antenv.axon_hooks with a working ctypes NTFF hook if the
    image's antenv package lacks it (concourse imports it under trace=True)."""
    import sys as _sys

    if "antenv.axon_hooks" in _sys.modules:
        return
    try:
        from antenv import axon_hooks  # noqa: F401

        return
    except ImportError:
        pass
    import contextlib
    import ctypes
    import types

    so_path = "/opt/axon/libaxon_pjrt.so"
    hook = None
    try:
        lib = ctypes.CDLL(so_path)
        if hasattr(lib, "axon_start_nrt_profile"):
            lib.axon_start_nrt_profile.argtypes = [
                ctypes.POINTER(ctypes.c_int64),
                ctypes.c_size_t,
            ]
            lib.axon_start_nrt_profile.restype = ctypes.c_int64
            lib.axon_stop_nrt_profile.argtypes = [ctypes.c_char_p]
            lib.axon_stop_nrt_profile.restype = ctypes.c_int64

            @contextlib.contextmanager
            def _hook(output_dir, device_ids):
                import jax

                jax.devices()
                if device_ids:
                    ids = (ctypes.c_int64 * len(device_ids))(*device_ids)
                    rc = lib.axon_start_nrt_profile(ids, len(device_ids))
                else:
                    rc = lib.axon_start_nrt_profile(None, 0)
                if rc != 0:
                    raise RuntimeError(f"axon_start_nrt_profile rc={rc}")
                try:
                    yield
                finally:
                    n = lib.axon_stop_nrt_profile(str(output_dir).encode())
                    print(f"profile: {n} file(s) written to {output_dir}")

            hook = _hook
    except OSError:
        pass

    mod = types.ModuleType("antenv.axon_hooks")
    mod._hook = hook
    mod.get_axon_ntff_profile_hook = lambda: mod._hook

    def _set(h):
        mod._hook = h

    mod.set_axon_ntff_profile_hook = _set
    _sys.modules["antenv.axon_hooks"] = mod


def _build_nc():
    import concourse.bacc as bacc
    import concourse.tile as tile
    from concourse import mybir
    from contextlib import ExitStack

    f32 = mybir.dt.float32
    f32r = mybir.dt.float32r
    AX = mybir.AxisListType
    OP = mybir.AluOpType
    AF = mybir.ActivationFunctionType

    def mmdt(ap):
        return ap.bitcast(f32r) if _F32R else ap

    nc = bacc.Bacc("TRN2", target_bir_lowering=False, debug=False)

    # ---- DRAM I/O ----
    ctxT_d = nc.dram_tensor("ctxT", [BPC, EJ, P, C], f32, kind="ExternalInput").ap()
    ctxN_d = nc.dram_tensor("ctxN", [BPC, CK, P, E], f32, kind="ExternalInput").ap()
    qT_d = nc.dram_tensor("qT", [BPC, EJ, P, Q], f32, kind="ExternalInput").ap()
    qN_d = nc.dram_tensor("qN", [BPC, Q, E], f32, kind="ExternalInput").ap()
    wq_d = nc.dram_tensor("wq", [EJ, P], f32, kind="ExternalInput").ap()
    wc_d = nc.dram_tensor("wc", [EJ, P], f32, kind="ExternalInput").ap()
    wm_d = nc.dram_tensor("wm", [EJ, P], f32, kind="ExternalInput").ap()
    ident_d = nc.dram_tensor("ident", [P, P], f32, kind="ExternalInput").ap()
    c2qT_o = nc.dram_tensor("c2qT_out", [BPC, P, EJ, C], f32, kind="ExternalOutput").ap()
    q2c_o = nc.dram_tensor("q2c_out", [2, 4, E], f32, kind="ExternalOutput").ap()
    dbg = {}
    if _DEBUG:
        for nm, shp in [
            ("dbg_mx", [P, BPC, CK]),
            ("dbg_cw", [P, BPC, CK]),
            ("dbg_sm", [P, BPC, CK]),
            ("dbg_u", [P, BPC, CK]),
            ("dbg_qwrow", [1, BPC, Q]),
            ("dbg_qwcol", [Q, BPC]),
            ("dbg_sht", [BPC * CK, P]),
            ("dbg_shift2t", [P, 2, C]),
            ("dbg_ztn", [Q, C]),
        ]:
            dbg[nm] = nc.dram_tensor(nm, shp, f32, kind="ExternalOutput").ap()

    with tile.TileContext(nc) as tc, ExitStack() as ctx:
        singles = ctx.enter_context(tc.tile_pool(name="singles", bufs=1))
        zpool = ctx.enter_context(tc.tile_pool(name="zpool", bufs=2))
        ztpool = ctx.enter_context(tc.tile_pool(name="ztpool", bufs=2))
        stpool = ctx.enter_context(tc.tile_pool(name="stage", bufs=2))
        ps_stats = ctx.enter_context(tc.tile_pool(name="ps_stats", bufs=2, space="PSUM"))
        ps_small = ctx.enter_context(tc.tile_pool(name="ps_small", bufs=1, space="PSUM"))
        ps_work = ctx.enter_context(tc.tile_pool(name="ps_work", bufs=3, space="PSUM"))
        ps_q2c = ctx.enter_context(tc.tile_pool(name="ps_q2c", bufs=1, space="PSUM"))

        # ---- constants / small inputs ----
        ident = singles.tile([P, P], f32)
        nc.sync.dma_start(out=ident, in_=ident_d)
        wq_c = singles.tile([P, EJ], f32)
        nc.sync.dma_start(out=wq_c, in_=wq_d.rearrange("j p -> p j"))
        wc_c = singles.tile([P, EJ], f32)
        nc.sync.dma_start(out=wc_c, in_=wc_d.rearrange("j p -> p j"))
        wm_c = singles.tile([P, EJ], f32)
        nc.sync.dma_start(out=wm_c, in_=wm_d.rearrange("j p -> p j"))
        ones_row = singles.tile([1, P], f32)
        nc.vector.memset(ones_row, 1.0)
        neg_rows = singles.tile([P, Q], f32)  # -1 rows at any 32-aligned base
        nc.vector.memset(neg_rows, -1.0)
        ones_col = singles.tile([P, 1], f32)
        nc.vector.memset(ones_col, 1.0)
        neg_shift = singles.tile([P, 1], f32)
        nc.vector.memset(neg_shift, -Q2C_SHIFT)

        qT_sb = []
        qN_sb = []
        rhs65 = []  # [:, j, 0:64] = qmT chunk, [:, j, 64] = wc chunk
        for b in range(BPC):
            t = singles.tile([P, EJ, Q], f32, name=f"qT{b}")
            nc.sync.dma_start(out=t, in_=qT_d[b].rearrange("j p q -> p j q"))
            qT_sb.append(t)
            t2 = singles.tile([Q, E], f32, name=f"qN{b}")
            nc.sync.dma_start(out=t2, in_=qN_d[b])
            qN_sb.append(t2)
            r = singles.tile([P, EJ, Q + 1], f32, name=f"rhs65_{b}")
            for j in range(EJ):
                nc.vector.tensor_scalar_mul(
                    out=r[:, j, 0:Q], in0=t[:, j, :], scalar1=wm_c[:, j : j + 1]
                )
                nc.vector.tensor_copy(out=r[:, j, Q : Q + 1], in_=wc_c[:, j : j + 1])
            rhs65.append(r)

        # qw as a row (1, b, 64) and as columns (64, b); sequential through
        # one psum bank (ps_small has a single slot)
        qw_row_ps = ps_small.tile([1, BPC, Q], f32, name="small")
        for b in range(BPC):
            for j in range(EJ):
                nc.tensor.matmul(
                    out=qw_row_ps[0:1, b, :],
                    lhsT=wq_c[:, j : j + 1],
                    rhs=qT_sb[b][:, j, :],
                    start=(j == 0),
                    stop=(j == EJ - 1),
                )
        # 65-wide row with a zero in the cw column, so the rank-1 broadcast
        # matmul can target the full (contiguous) stats psum tile
        qw_row = singles.tile([1, BPC, Q + 1], f32)
        nc.vector.memset(qw_row, 0.0)
        nc.vector.tensor_copy(out=qw_row[:, :, 0:Q], in_=qw_row_ps)
        qw_col_ps = ps_small.tile([Q, BPC], f32, name="small")
        for b in range(BPC):
            for j in range(EJ):
                nc.tensor.matmul(
                    out=qw_col_ps[:, b : b + 1],
                    lhsT=qT_sb[b][:, j, :],
                    rhs=wq_c[:, j : j + 1],
                    start=(j == 0),
                    stop=(j == EJ - 1),
                )
        qw_col = singles.tile([Q, BPC], f32)
        nc.vector.tensor_copy(out=qw_col, in_=qw_col_ps)

        # ---- context loads (transposed layout) ----
        ctxT_sb = []
        for b in range(BPC):
            t = singles.tile([P, EJ, C], f32, name=f"ctxT{b}")
            nc.sync.dma_start(out=t, in_=ctxT_d[b].rearrange("j p c -> p j c"))
            ctxT_sb.append(t)

        # ---- phase 1: stats orientation ----
        HB = BPC // 2  # 4 batches per psum tile
        mx_all = singles.tile([P, BPC, CK], f32)  # rowmax(sim') per (c, b, k)
        sm_all = singles.tile([P, BPC, CK], f32)  # rowsum(exp(sim'-mx))
        cw_all = singles.tile([P, BPC, CK], f32)  # ctx @ w_c

        for k in range(CK):
            ps_halves = []
            for h in range(2):
                ps = ps_stats.tile([P, HB, Q + 1], f32, name="stats")
                ps_halves.append(ps)
                # qw broadcast over the sim columns goes FIRST (rank-1 over
                # the whole contiguous tile, start=True clears it; the 65th
                # column seeds cw with 0) -- the per-batch matmuls then
                # accumulate on top.
                nc.tensor.matmul(
                    out=ps,
                    lhsT=ones_row,
                    rhs=qw_row[0:1, h * HB : (h + 1) * HB, :],
                    start=True,
                    stop=False,
                    skip_group_check=True,
                )
                for bb in range(HB):
                    b = h * HB + bb
                    for j in range(EJ):
                        nc.tensor.matmul(
                            out=ps[:, bb, :],
                            lhsT=mmdt(ctxT_sb[b][:, j, k * P : (k + 1) * P]),
                            rhs=mmdt(rhs65[b][:, j, :]),
                            start=False,
                            stop=(j == EJ - 1),
                            skip_group_check=True,
                        )
                nc.vector.reduce_max(
                    out=mx_all[:, h * HB : (h + 1) * HB, k : k + 1],
                    in_=ps[:, :, 0:Q],
                    axis=AX.X,
                )
                nc.vector.tensor_copy(
                    out=cw_all[:, h * HB : (h + 1) * HB, k : k + 1],
                    in_=ps[:, :, Q : Q + 1],
                )
            z = zpool.tile([P, BPC, Q], f32, name="z")
            for h in range(2):
                nc.vector.tensor_tensor(
                    out=z[:, h * HB : (h + 1) * HB, :],
                    in0=ps_halves[h][:, :, 0:Q],
                    in1=mx_all[:, h * HB : (h + 1) * HB, k : k + 1].broadcast_to(
                        (P, HB, Q)
                    ),
                    op=OP.subtract,
                )
            nc.scalar.activation(out=z, in_=z, func=AF.Exp)
            nc.vector.reduce_sum(
                out=sm_all[:, :, k : k + 1], in_=z, axis=AX.X
            )

        # ---- phase 1b: shift2 rows + q2c weights ----
        lnsm = singles.tile([P, BPC, CK], f32)
        nc.scalar.activation(out=lnsm, in_=sm_all, func=AF.Ln)
        shift2 = singles.tile([P, BPC * CK], f32)
        nc.vector.tensor_tensor(
            out=shift2.rearrange("p (b k) -> p b k", b=BPC),
            in0=mx_all,
            in1=lnsm,
            op=OP.add,
        )
        m_all = singles.tile([P, BPC, CK], f32)
        nc.vector.tensor_tensor(out=m_all, in0=mx_all, in1=cw_all, op=OP.add)
        # u = exp(m - SHIFT): unnormalized q2c softmax weights, column form
        u_all = singles.tile([P, BPC, CK], f32)
        nc.scalar.activation(
            out=u_all, in_=m_all, func=AF.Exp, bias=neg_shift[:, 0:1], scale=1.0
        )

        # shift2 (P, b*k) --PE transpose--> (b*k, P) --reshape DMA--> (b, k*P)
        sh_t_ps = ps_small.tile([BPC * CK, P], f32, name="small")
        nc.tensor.transpose(out=sh_t_ps, in_=shift2, identity=ident)
        sh_t = singles.tile([BPC * CK, P], f32)
        nc.vector.tensor_copy(out=sh_t, in_=sh_t_ps)
        # batch b = g*4 + a lands on partition 32*a of column group g, so
        # every matmul rhs slice starts on a 32-aligned partition
        shift2t = singles.tile([P, 2, C], f32)
        for g in range(2):
            nc.sync.dma_start(
                out=shift2t.rearrange("(a r) g (k p) -> a r g k p", r=32, k=CK)[
                    :, 0, g, :, :
                ],
                in_=sh_t[g * 32 : (g + 1) * 32, :],
            )

        if _DEBUG:
            nc.sync.dma_start(out=dbg["dbg_mx"], in_=mx_all)
            nc.sync.dma_start(out=dbg["dbg_cw"], in_=cw_all)
            nc.sync.dma_start(out=dbg["dbg_sm"], in_=sm_all)
            nc.sync.dma_start(out=dbg["dbg_u"], in_=u_all)
            nc.sync.dma_start(out=dbg["dbg_qwrow"], in_=qw_row[:, :, 0:Q])
            nc.sync.dma_start(out=dbg["dbg_qwcol"], in_=qw_col)
            nc.sync.dma_start(out=dbg["dbg_sht"], in_=sh_t)
            nc.sync.dma_start(out=dbg["dbg_shift2t"], in_=shift2t)

        # ---- natural-layout context loads (for q2c) ----
        ctxN_sb = []
        for b in range(BPC):
            t = singles.tile([P, CK, E], f32, name=f"ctxN{b}")
            nc.sync.dma_start(out=t, in_=ctxN_d[b].rearrange("k p e -> p k e"))
            ctxN_sb.append(t)

        # ---- phase 2: transposed softmax + c2qT ----
        NC2 = C // 512
        for b in range(BPC):
            ztn = ztpool.tile([Q, NC2, 512], f32, name="ztn")
            for n2 in range(NC2):
                zt_ps = ps_work.tile([Q, 512], f32, name="work")
                for j in range(EJ):
                    nc.tensor.matmul(
                        out=zt_ps,
                        lhsT=mmdt(rhs65[b][:, j, 0:Q]),
                        rhs=mmdt(ctxT_sb[b][:, j, n2 * 512 : (n2 + 1) * 512]),
                        start=(j == 0),
                        stop=False,
                    )
                g, a = divmod(b, 4)
                nc.tensor.matmul(
                    out=zt_ps,
                    lhsT=mmdt(neg_rows[32 * a : 32 * a + 1, :]),
                    rhs=mmdt(
                        shift2t[32 * a : 32 * a + 1, g, n2 * 512 : (n2 + 1) * 512]
                    ),
                    start=False,
                    stop=True,
                    skip_group_check=True,
                    tile_position=(32 * a, 0),
                )
                # ZTn = exp(simT - shift2 + qw) = softmax^T directly
                nc.scalar.activation(
                    out=ztn[:, n2, :],
                    in_=zt_ps,
                    func=AF.Exp,
                    bias=qw_col[:, b : b + 1],
                    scale=1.0,
                )
            if _DEBUG and b == 0:
                nc.sync.dma_start(
                    out=dbg["dbg_ztn"].rearrange("q (n c) -> q n c", n=NC2), in_=ztn
                )
            stage = stpool.tile([P, EJ, C], f32, name="stage")
            for je in range(EJ):
                for n2 in range(NC2):
                    po = ps_work.tile([P, 512], f32, name="work")
                    nc.tensor.matmul(
                        out=po,
                        lhsT=mmdt(qN_sb[b][:, je * P : (je + 1) * P]),
                        rhs=mmdt(ztn[:, n2, :]),
                        start=True,
                        stop=True,
                    )
                    if n2 == 0:
                        nc.vector.tensor_copy(
                            out=stage[:, je, n2 * 512 : (n2 + 1) * 512], in_=po
                        )
                    else:
                        nc.scalar.copy(
                            out=stage[:, je, n2 * 512 : (n2 + 1) * 512], in_=po
                        )
            nc.sync.dma_start(out=c2qT_o[b], in_=stage)

        # ---- q2c ----
        upart = singles.tile([P, BPC], f32)
        nc.vector.reduce_sum(out=upart, in_=u_all, axis=AX.X)
        q2c_ps = [ps_q2c.tile([P, E + 1], f32, name=f"q2cp{t}") for t in range(2)]
        for b in range(BPC):
            t, jj = divmod(b, 4)
            for k in range(CK):
                nc.tensor.matmul(
                    out=q2c_ps[t][32 * jj : 32 * jj + 1, 0:E],
                    lhsT=mmdt(u_all[:, b, k : k + 1]),
                    rhs=mmdt(ctxN_sb[b][:, k, :]),
                    start=(k == 0),
                    stop=(k == CK - 1),
                    tile_position=(0, 32 * jj),
                )
            nc.tensor.matmul(
                out=q2c_ps[t][32 * jj : 32 * jj + 1, E : E + 1],
                lhsT=upart[:, b : b + 1],
                rhs=ones_col,
                start=True,
                stop=True,
                tile_position=(0, 32 * jj),
            )
        for t in range(2):
            rec = singles.tile([P, 1], f32, name=f"rec{t}")
            nc.vector.reciprocal(out=rec, in_=q2c_ps[t][:, E : E + 1])
            q2c_sb = singles.tile([P, E], f32, name=f"q2c_sb{t}")
            nc.vector.tensor_scalar_mul(
                out=q2c_sb, in0=q2c_ps[t][:, 0:E], scalar1=rec
            )
            nc.sync.dma_start(
                out=q2c_o[t],
                in_=q2c_sb.rearrange("(a b) e -> a b e", b=32)[:, 0, :],
            )

    nc.compile()
    return nc


@functools.lru_cache(maxsize=1)
def _get_nc():
    return _build_nc()


def kernel(context, question, ws):
    global LAST_EXEC_NS, LAST_TRACE
    from concourse import bass_utils

    context = np.asarray(context, dtype=np.float32)
    question = np.asarray(question, dtype=np.float32)
    ws = np.asarray(ws, dtype=np.float32)
    w_q, w_c, w_m = ws[:E], ws[E : 2 * E], ws[2 * E :]

    nc = _get_nc()

    in_maps = []
    for i in range(NCORES):
        cs = slice(i * BPC, (i + 1) * BPC)
        ctx_i = context[cs]  # (BPC, C, E)
        q_i = question[cs]  # (BPC, Q, E)
        in_maps.append(
            {
                "ctxT": np.ascontiguousarray(
                    ctx_i.transpose(0, 2, 1).reshape(BPC, EJ, P, C)
                ),
                "ctxN": np.ascontiguousarray(ctx_i.reshape(BPC, CK, P, E)),
                "qT": np.ascontiguousarray(
                    q_i.transpose(0, 2, 1).reshape(BPC, EJ, P, Q)
                ),
                "qN": np.ascontiguousarray(q_i),
                "wq": np.ascontiguousarray(w_q.reshape(EJ, P)),
                "wc": np.ascontiguousarray(w_c.reshape(EJ, P)),
                "wm": np.ascontiguousarray(w_m.reshape(EJ, P)),
                "ident": np.eye(P, dtype=np.float32),
            }
        )

    trace = os.environ.get("KERNEL_TRACE", "0") == "1"
    if trace:
        _ensure_ntff_hook()
    res = bass_utils.run_bass_kernel_spmd(
        nc, in_maps, core_ids=list(range(NCORES)), trace=trace
    )
    LAST_EXEC_NS = res.exec_time_ns
    LAST_TRACE = res.instructions_and_trace[1] if res.instructions_and_trace else None

    c2q = np.empty((B, C, E), dtype=np.float32)
    q2c = np.empty((B, 1, E), dtype=np.float32)
    for i in range(NCORES):
        r = res.results[i]
        # c2qT_out: (BPC, P, EJ, C) with [b, p, j, c] = c2q[b, c, j*128+p]
        arr = r["c2qT_out"].transpose(0, 3, 2, 1).reshape(BPC, C, E)
        c2q[i * BPC : (i + 1) * BPC] = arr
        q2c[i * BPC : (i + 1) * BPC, 0, :] = r["q2c_out"].reshape(BPC, E)
    return (c2q, q2c)
